# revision 14
# baseline (speedup 1.0000x reference)
"""Causal self-attention layer (LN + QKV + RoPE + GQA attention + proj) on 8 trn2 cores.

Sharding: sequence-parallel with pairwise K/V exchange. 8 cores = 4 packed
sequences x 2 query-halves. Core c=(s,h) owns query rows [h*512, h*512+512)
of sequence s: it normalizes and projects only its own 512 tokens, then the
pair of cores of each sequence AllGathers K/V halves (global key order) so
each core attends over the full sequence. Causality is enforced by per-core
mask data: an additive exp-bias column zeroes fully-masked key rows and
narrow leading-window mask multiplies handle the diagonal triangles, so the
single SPMD program is uniform across cores.

All matmuls run in bf16 with fp32 PSUM accumulation. Weights are pre-tiled on
the host (two k-steps per DMA) so every weight DMA is one contiguous block.
The xn transpose runs on the PE (is_transpose matmul). The softmax
denominator uses a [128,128] ones stationary so its matmul output is already
partition-broadcast.
"""

import os
import sys
import numpy as np

try:
    import concourse.bass as bass  # noqa: F401
except Exception:  # pragma: no cover
    for p in ("/opt/trn_rl_repo", "/root/.axon_site/_ro/trn_rl_repo"):
        if os.path.isdir(p) and p not in sys.path:
            sys.path.insert(0, p)

import ml_dtypes
import concourse.bass as bass
import concourse.tile as tile
from concourse import bacc, mybir
from concourse.bass_utils import run_bass_kernel_spmd

F32 = mybir.dt.float32
BF16 = mybir.dt.bfloat16

CFG_FULL = dict(H=4096, NQ=32, NKV=8, D=128, S=1024, B=4)
BASE = 10000.0
EPS = 1e-5
MASK_NEG = -30000.0

LAST_EXEC_NS = None
LAST_RESULT = None


def _ceil_div(a, b):
    return (a + b - 1) // b


def _geom(cfg):
    H, NQ, NKV, D, S = cfg["H"], cfg["NQ"], cfg["NKV"], cfg["D"], cfg["S"]
    g = {}
    g["RQ"] = S // 2
    g["RK"] = S
    g["HT"] = H // 128
    g["NT_K"] = S // 128
    g["NT_Q"] = g["RQ"] // 128
    g["NT_O"] = g["RQ"] // 128       # own token tiles
    g["VC"] = NKV * D
    g["REP"] = NQ // NKV
    g["GQ"] = 4
    g["NGQ"] = NQ // g["GQ"]
    g["GK"] = 4
    g["NGK"] = NKV // g["GK"]
    g["VCH"] = min(512, g["VC"])
    g["NCV"] = g["VC"] // g["VCH"]
    g["gcols"] = 512
    g["NGP"] = H // g["gcols"]
    return g


def build_bass(cfg):
    """Build the single-core SPMD program (identical across cores)."""
    H, NQ, NKV, D, S = cfg["H"], cfg["NQ"], cfg["NKV"], cfg["D"], cfg["S"]
    assert D == 128
    g = _geom(cfg)
    RQ, RK, HT, NT_K, NT_Q = g["RQ"], g["RK"], g["HT"], g["NT_K"], g["NT_Q"]
    NT_O = g["NT_O"]
    VC, REP = g["VC"], g["REP"]
    GQ, NGQ, GK, NGK = g["GQ"], g["NGQ"], g["GK"], g["NGK"]
    VCH, NCV, gcols, NGP = g["VCH"], g["NCV"], g["gcols"], g["NGP"]
    assert HT >= NQ and RK >= RQ
    HT2, NQ2 = HT // 2, NQ // 2

    nc = bacc.Bacc(None, target_bir_lowering=False, num_devices=8)

    x_d = nc.dram_tensor("x", [RQ, H], BF16, kind="ExternalInput")
    # weights pre-tiled on host: [k-pair, group, 128, 2, cols] so one DMA
    # loads two k-steps of one group as contiguous 2KB partition lines
    wq_d = nc.dram_tensor("wq", [HT2, NGQ, 128, 2, GQ * 128], BF16,
                          kind="ExternalInput")
    wk_d = nc.dram_tensor("wk", [HT2, NGK, 128, 2, GK * 128], BF16,
                          kind="ExternalInput")
    wv_d = nc.dram_tensor("wv", [HT2, NCV, 128, 2, VCH], BF16,
                          kind="ExternalInput")
    wp_d = nc.dram_tensor("wp", [NQ2, NGP, 128, 2, gcols], BF16,
                          kind="ExternalInput")
    bq_d = nc.dram_tensor("bq", [128, NQ], F32, kind="ExternalInput")
    bk_d = nc.dram_tensor("bk", [128, NKV], F32, kind="ExternalInput")
    bv_d = nc.dram_tensor("bv", [1, VC], F32, kind="ExternalInput")
    bp_d = nc.dram_tensor("bp", [1, H], F32, kind="ExternalInput")
    cq_d = nc.dram_tensor("cq", [64, RQ], F32, kind="ExternalInput")
    sq_d = nc.dram_tensor("sq", [64, RQ], F32, kind="ExternalInput")
    ck_d = nc.dram_tensor("ck", [64, RQ], F32, kind="ExternalInput")
    sk_d = nc.dram_tensor("sk", [64, RQ], F32, kind="ExternalInput")
    mask_d = nc.dram_tensor("mask", [RK, RQ], BF16, kind="ExternalInput")
    ebias_d = nc.dram_tensor("ebias", [128, NT_K], F32, kind="ExternalInput")
    ident_d = nc.dram_tensor("ident", [128, 128], BF16, kind="ExternalInput")
    out_d = nc.dram_tensor("out", [RQ, H], F32, kind="ExternalOutput")

    PAIRS = [[0, 1], [2, 3], [4, 5], [6, 7]]

    def dmae(i):
        # alternate DMA issue between the two HWDGE engines
        return nc.sync if i % 2 == 0 else nc.scalar

    n_sub = _ceil_div(H, 512)
    sub = H // n_sub
    assert sub * n_sub == H and sub <= 512

    with tile.TileContext(nc) as tc:
        with (
            tc.tile_pool(name="const", bufs=1) as const,
            tc.tile_pool(name="wstream", bufs=3) as wstream,
            tc.tile_pool(name="dram", bufs=1, space="DRAM") as dram,
        ):
            # ---- constants ----
            cq_sb = const.tile([64, RQ], F32, tag="cq")
            sq_sb = const.tile([64, RQ], F32, tag="sq")
            nc.sync.dma_start(out=cq_sb[:], in_=cq_d[:])
            nc.scalar.dma_start(out=sq_sb[:], in_=sq_d[:])
            bq_sb = const.tile([128, NQ], F32, tag="bq")
            bk_sb = const.tile([128, NKV], F32, tag="bk")
            nc.sync.dma_start(out=bq_sb[:], in_=bq_d[:])
            nc.scalar.dma_start(out=bk_sb[:], in_=bk_d[:])
            ident_sb = const.tile([128, 128], BF16, tag="ident")
            nc.sync.dma_start(out=ident_sb[:], in_=ident_d[:])
            ones_blk = const.tile([128, 128], BF16, tag="ones_blk")
            nc.vector.memset(ones_blk[:], 1.0)
            eps_sb = const.tile([128, 1], F32, tag="eps")
            nc.vector.memset(eps_sb[:], EPS)

            # DRAM bounce buffers for the pairwise K/V AllGather
            kv_src = dram.tile([128, 2 * NKV, 512], BF16, tag="kv_src")
            kv_dst = dram.tile([2, 128, 2 * NKV, 512], BF16, tag="kv_dst")

            with tc.tile_pool(name="xnt_pool", bufs=1) as xnt_pool:
                xnT = xnt_pool.tile([128, HT, RQ], BF16, tag="xnT")

                with tc.tile_pool(name="qkvout", bufs=1) as qkvout:
                    QT = qkvout.tile([128, NQ, RQ], BF16, tag="QT")
                    KT = qkvout.tile([128, NKV, RK], BF16, tag="KT")
                    Vn = qkvout.tile([128, NT_K, VC], BF16, tag="Vn")

                    # ---- LN + PE-transpose of the own 512 tokens ----
                    with (
                        tc.tile_pool(name="ln", bufs=2) as ln_pool,
                        tc.tile_pool(name="stat", bufs=3) as stat,
                        tc.tile_pool(name="tps", bufs=2, space="PSUM") as tps,
                    ):
                        for tt in range(NT_O):
                            xt = ln_pool.tile([128, H], BF16, tag="xt")
                            dmae(tt).dma_start(
                                out=xt[:], in_=x_d[tt * 128:(tt + 1) * 128, :])
                            stats = stat.tile([128, n_sub, 6], F32,
                                              tag="stats")
                            xt3 = xt[:].rearrange("p (n f) -> p n f", f=sub)
                            for si in range(n_sub):
                                nc.vector.bn_stats(out=stats[:, si, :],
                                                   in_=xt3[:, si, :])
                            mv = stat.tile([128, 2], F32, tag="mv")
                            nc.vector.bn_aggr(out=mv[:], in_=stats[:])
                            rstd = stat.tile([128, 1], F32, tag="rstd")
                            nc.scalar.activation(
                                out=rstd[:], in_=mv[:, 1:2],
                                func=mybir.ActivationFunctionType.Sqrt,
                                bias=eps_sb[:], scale=1.0,
                            )
                            nc.vector.reciprocal(out=rstd[:], in_=rstd[:])
                            xnt = ln_pool.tile([128, H], BF16, tag="xnt")
                            nc.vector.tensor_scalar(
                                out=xnt[:], in0=xt[:],
                                scalar1=mv[:, 0:1], scalar2=rstd[:],
                                op0=mybir.AluOpType.subtract,
                                op1=mybir.AluOpType.mult,
                            )
                            for hg in range(HT // 4):
                                tp = tps.tile([128, 4, 128], BF16, tag="tp")
                                for i in range(4):
                                    ht = hg * 4 + i
                                    nc.tensor.transpose(
                                        tp[:, i, :],
                                        xnt[:, ht * 128:(ht + 1) * 128],
                                        ident_sb[:],
                                    )
                                nc.scalar.copy(
                                    out=xnT[:, hg * 4:(hg + 1) * 4,
                                            tt * 128:(tt + 1) * 128],
                                    in_=tp[:],
                                )

                    # -- K (own tokens): KT[:, :, 0:512] --
                    with (
                        tc.tile_pool(name="ropek", bufs=2) as ropek,
                        tc.tile_pool(name="ropet2", bufs=1) as ropet2,
                        tc.tile_pool(name="ps_k", bufs=8,
                                     space="PSUM") as ps_k,
                    ):
                        ck_sb = ropek.tile([64, RQ], F32, tag="ck", bufs=1)
                        sk_sb = ropek.tile([64, RQ], F32, tag="sk", bufs=1)
                        nc.sync.dma_start(out=ck_sb[:], in_=ck_d[:])
                        nc.scalar.dma_start(out=sk_sb[:], in_=sk_d[:])

                        def rope_apply(dst, lo, hi, cos_sb, sin_sb, tpool):
                            t1 = tpool.tile([64, 512], F32, tag="t1")
                            t2 = tpool.tile([64, 512], F32, tag="t2")
                            nc.vector.tensor_mul(t1[:], hi[:], sin_sb[:])
                            nc.vector.tensor_mul(t2[:], lo[:], cos_sb[:])
                            nc.vector.tensor_sub(dst[0:64, :], t2[:], t1[:])
                            t3 = tpool.tile([64, 512], F32, tag="t1")
                            t4 = tpool.tile([64, 512], F32, tag="t2")
                            nc.vector.tensor_mul(t3[:], lo[:], sin_sb[:])
                            nc.vector.tensor_mul(t4[:], hi[:], cos_sb[:])
                            nc.vector.tensor_add(dst[64:128, :], t4[:], t3[:])

                        def pevac(psum_ap, bias_col, lo, hi):
                            # evacuate psum halves; hi half realigned to
                            # partition 0 (two-SBUF-input ops need equal
                            # input base partitions)
                            nc.scalar.activation(
                                out=lo[:], in_=psum_ap[0:64, :],
                                func=mybir.ActivationFunctionType.Identity,
                                bias=bias_col[0:64], scale=1.0,
                            )
                            nc.scalar.activation(
                                out=hi[:], in_=psum_ap[64:128, :],
                                func=mybir.ActivationFunctionType.Identity,
                                bias=bias_col[64:128], scale=1.0,
                            )

                        for gidx in range(NGK):
                            psk = [ps_k.tile([128, RQ], F32, tag="ps",
                                             name=f"psk{gi}")
                                   for gi in range(GK)]
                            for k2 in range(HT2):
                                wb = wstream.tile([128, 2, GK * 128],
                                                  BF16, tag="wk")
                                dmae(k2).dma_start(
                                    out=wb[:], in_=wk_d[k2, gidx])
                                for j in range(2):
                                    for gi in range(GK):
                                        nc.tensor.matmul(
                                            psk[gi][:],
                                            wb[:, j, gi * 128:(gi + 1) * 128],
                                            xnT[:, k2 * 2 + j, :],
                                            start=(k2 == 0 and j == 0),
                                            stop=(k2 == HT2 - 1 and j == 1),
                                        )
                            for gi in range(GK):
                                h = gidx * GK + gi
                                klo = ropek.tile([64, RQ], F32, tag="klo")
                                khi = ropek.tile([64, RQ], F32, tag="khi")
                                pevac(psk[gi][:], bk_sb[:, h:h + 1], klo, khi)
                                rope_apply(KT[:, h, 0:RQ], klo, khi,
                                           ck_sb, sk_sb, ropet2)

                    # -- V (own tokens): Vn[:, 0:4, :] --
                    with (
                        tc.tile_pool(name="vb", bufs=1) as vb,
                        tc.tile_pool(name="ps_v", bufs=8,
                                     space="PSUM") as ps_v,
                    ):
                        bv_sb = vb.tile([128, VC], F32, tag="bv")
                        nc.gpsimd.dma_start(
                            out=bv_sb[:],
                            in_=bass.AP(tensor=bv_d, offset=0,
                                        ap=[[0, 128], [1, VC]]),
                        )
                        for vch in range(NCV):
                            c0 = vch * VCH
                            psv = [ps_v.tile([128, VCH], F32, tag="ps",
                                             name=f"psv{ti}")
                                   for ti in range(NT_O)]
                            for k2 in range(HT2):
                                wb = wstream.tile([128, 2, VCH], BF16,
                                                  tag="wv")
                                dmae(k2).dma_start(
                                    out=wb[:], in_=wv_d[k2, vch])
                                for j in range(2):
                                    for tt in range(NT_O):
                                        nc.tensor.matmul(
                                            psv[tt][:],
                                            xnT[:, k2 * 2 + j,
                                                tt * 128:(tt + 1) * 128],
                                            wb[:, j, :],
                                            start=(k2 == 0 and j == 0),
                                            stop=(k2 == HT2 - 1 and j == 1),
                                        )
                            for tt in range(NT_O):
                                nc.vector.scalar_tensor_tensor(
                                    out=Vn[:, tt, c0:c0 + VCH],
                                    in0=psv[tt][:], scalar=1.0,
                                    in1=bv_sb[:, c0:c0 + VCH],
                                    op0=mybir.AluOpType.mult,
                                    op1=mybir.AluOpType.add,
                                )

                    # -- stage own K/V and AllGather with the pair core --
                    nc.sync.dma_start(out=kv_src[:, 0:NKV, :],
                                      in_=KT[:, :, 0:RQ])
                    nc.scalar.dma_start(
                        out=kv_src[:, NKV:2 * NKV, :],
                        in_=Vn[:, 0:NT_O, :].rearrange(
                            "p a (b c) -> p (a b) c", c=512),
                    )
                    nc.gpsimd.collective_compute(
                        "AllGather",
                        mybir.AluOpType.bypass,
                        replica_groups=PAIRS,
                        ins=[kv_src[:].opt()],
                        outs=[kv_dst[:].opt()],
                    )
                    # load back in global key order (uniform across cores);
                    # issued on gpsimd so the HWDGE queues stay free for the
                    # Q-phase weight stream below
                    nc.gpsimd.dma_start(out=KT[:, :, 0:RQ],
                                        in_=kv_dst[0, :, 0:NKV, :])
                    nc.gpsimd.dma_start(out=KT[:, :, RQ:RK],
                                        in_=kv_dst[1, :, 0:NKV, :])
                    nc.gpsimd.dma_start(
                        out=Vn[:, 0:NT_O, :].rearrange(
                            "p a (b c) -> p (a b) c", c=512),
                        in_=kv_dst[0, :, NKV:2 * NKV, :])
                    nc.gpsimd.dma_start(
                        out=Vn[:, NT_O:NT_K, :].rearrange(
                            "p a (b c) -> p (a b) c", c=512),
                        in_=kv_dst[1, :, NKV:2 * NKV, :])

                    # -- Q: QT[h] = wq[:,h].T @ xnT -- (hides the collective)
                    with (
                        tc.tile_pool(name="ropeq", bufs=2) as ropeq,
                        tc.tile_pool(name="ropet", bufs=1) as ropet,
                        tc.tile_pool(name="ps_q", bufs=8,
                                     space="PSUM") as ps_q,
                    ):
                        for gidx in range(NGQ):
                            psq = [ps_q.tile([128, RQ], F32, tag="ps",
                                             name=f"psq{gi}")
                                   for gi in range(GQ)]
                            for k2 in range(HT2):
                                wb = wstream.tile([128, 2, GQ * 128],
                                                  BF16, tag="wq")
                                dmae(k2).dma_start(
                                    out=wb[:], in_=wq_d[k2, gidx])
                                for j in range(2):
                                    for gi in range(GQ):
                                        nc.tensor.matmul(
                                            psq[gi][:],
                                            wb[:, j, gi * 128:(gi + 1) * 128],
                                            xnT[:, k2 * 2 + j, :],
                                            start=(k2 == 0 and j == 0),
                                            stop=(k2 == HT2 - 1 and j == 1),
                                        )
                            for gi in range(GQ):
                                h = gidx * GQ + gi
                                qlo = ropeq.tile([64, RQ], F32, tag="qlo")
                                qhi = ropeq.tile([64, RQ], F32, tag="qhi")
                                pevac(psq[gi][:], bq_sb[:, h:h + 1],
                                      qlo, qhi)
                                rope_apply(QT[:, h, :], qlo, qhi,
                                           cq_sb, sq_sb, ropet)

                    # ---- attention per q head ----
                    # attnT[h] lives in xnT's dead space: xnT[:, h, :]
                    with (
                        tc.tile_pool(name="att", bufs=3) as att,
                        tc.tile_pool(name="small", bufs=2) as small,
                        tc.tile_pool(name="msk", bufs=1) as msk,
                        tc.tile_pool(name="ps_s", bufs=3,
                                     space="PSUM") as ps_s,
                        tc.tile_pool(name="ps_o", bufs=2,
                                     space="PSUM") as ps_o,
                        tc.tile_pool(name="ps_d", bufs=2,
                                     space="PSUM") as ps_d,
                    ):
                        mask_sb = msk.tile([128, NT_K, RQ], BF16, tag="mask")
                        nc.sync.dma_start(
                            out=mask_sb[:],
                            in_=mask_d[:].rearrange("(t p) q -> p t q", p=128),
                        )
                        eb_sb = msk.tile([128, NT_K], F32, tag="ebias")
                        nc.scalar.dma_start(out=eb_sb[:], in_=ebias_d[:])
                        for h in range(NQ):
                            gkv = h // REP
                            et = att.tile([128, NT_K, RQ], BF16, tag="expT")
                            for kt in range(NT_K):
                                sps = ps_s.tile([128, RQ], F32, tag="s")
                                nc.tensor.matmul(
                                    sps[:],
                                    KT[:, gkv, kt * 128:(kt + 1) * 128],
                                    QT[:, h, :],
                                    start=True, stop=True,
                                )
                                # exp bias (per key row) zeroes rows beyond
                                # this core's last query
                                nc.scalar.activation(
                                    out=et[:, kt, :], in_=sps[:],
                                    func=mybir.ActivationFunctionType.Exp,
                                    bias=eb_sb[:, kt:kt + 1], scale=1.0,
                                )
                                # leading-window mask covers this key tile's
                                # diagonal triangle (mask data is all-ones
                                # where a core has no triangle here)
                                w = (kt + 1) * 128 if kt < NT_O \
                                    else (kt - NT_O + 1) * 128
                                nc.vector.tensor_mul(
                                    et[:, kt, 0:w], et[:, kt, 0:w],
                                    mask_sb[:, kt, 0:w])
                            ops_ = ps_o.tile([128, RQ], F32, tag="o")
                            for kt in range(NT_K):
                                nc.tensor.matmul(
                                    ops_[:],
                                    Vn[:, kt, gkv * D:(gkv + 1) * D],
                                    et[:, kt, :],
                                    start=(kt == 0), stop=(kt == NT_K - 1),
                                )
                            # denominator via a [128,128] ones stationary:
                            # every output row is the same column sum, so the
                            # partition broadcast is built in
                            dps = ps_d.tile([128, RQ], F32, tag="d")
                            for kt in range(NT_K):
                                nc.tensor.matmul(
                                    dps[:],
                                    ones_blk[:],
                                    et[:, kt, :],
                                    start=(kt == 0), stop=(kt == NT_K - 1),
                                )
                            rbc = small.tile([128, RQ], F32, tag="rbc")
                            nc.vector.reciprocal(out=rbc[:], in_=dps[:])
                            nc.vector.tensor_mul(
                                xnT[:, h, :], ops_[:], rbc[:])

                # ---- out = attnT.T @ wp + bp ----
                with (
                    tc.tile_pool(name="late", bufs=1) as late,
                    tc.tile_pool(name="outp", bufs=3) as outp,
                    tc.tile_pool(name="ps_c", bufs=8, space="PSUM") as ps_c,
                ):
                    bp_sb = late.tile([128, H], F32, tag="bp")
                    nc.gpsimd.dma_start(
                        out=bp_sb[:],
                        in_=bass.AP(tensor=bp_d, offset=0,
                                    ap=[[0, 128], [1, H]]),
                    )
                    for gp in range(NGP):
                        n0 = gp * gcols
                        psc = [ps_c.tile([128, gcols], F32, tag="c",
                                         name=f"psc{qt}")
                               for qt in range(NT_Q)]
                        for k2 in range(NQ2):
                            wb = wstream.tile([128, 2, gcols], BF16, tag="wp")
                            dmae(k2).dma_start(out=wb[:], in_=wp_d[k2, gp])
                            for j in range(2):
                                for qt in range(NT_Q):
                                    nc.tensor.matmul(
                                        psc[qt][:],
                                        xnT[:, k2 * 2 + j,
                                            qt * 128:(qt + 1) * 128],
                                        wb[:, j, :],
                                        start=(k2 == 0 and j == 0),
                                        stop=(k2 == NQ2 - 1 and j == 1),
                                    )
                        for qt in range(NT_Q):
                            ot = outp.tile([128, gcols], F32, tag="ot")
                            nc.vector.scalar_tensor_tensor(
                                out=ot[:], in0=psc[qt][:], scalar=1.0,
                                in1=bp_sb[:, n0:n0 + gcols],
                                op0=mybir.AluOpType.mult,
                                op1=mybir.AluOpType.add,
                            )
                            dmae(gp + qt).dma_start(
                                out=out_d[qt * 128:(qt + 1) * 128,
                                          n0:n0 + gcols],
                                in_=ot[:],
                            )

    nc.finalize()  # bacc register allocation; the pjrt path serializes as-is
    return nc


def prep_core_inputs(cfg, c, hidden, ln_g, ln_b, w_qkv, b_qkv, w_proj, b_proj,
                     shared):
    """Per-core input dict. `shared` caches the weight prep across cores."""
    H, NQ, NKV, D, S = cfg["H"], cfg["NQ"], cfg["NKV"], cfg["D"], cfg["S"]
    g = _geom(cfg)
    RQ, NT_K = g["RQ"], g["NT_K"]
    if not shared:
        ln_g = np.asarray(ln_g, np.float32)
        ln_b = np.asarray(ln_b, np.float32)
        w_qkv = np.asarray(w_qkv, np.float32)
        b_qkv = np.asarray(b_qkv, np.float32)
        w_eff = ln_g[:, None] * w_qkv
        b_eff = b_qkv + ln_b @ w_qkv
        nqd, nkd = NQ * D, NKV * D
        HT, GQ, NGQ = g["HT"], g["GQ"], g["NGQ"]
        GK, NGK, VCH, NCV = g["GK"], g["NGK"], g["VCH"], g["NCV"]
        gcols, NGP = g["gcols"], g["NGP"]

        def tile_w(w, groups, gw):
            # [H, cols] -> [HT/2, groups, 128, 2, gw] contiguous blocks
            return np.ascontiguousarray(
                w.reshape(HT // 2, 2, 128, groups, gw).transpose(0, 3, 2, 1, 4)
            ).astype(ml_dtypes.bfloat16)

        shared["wq"] = tile_w(w_eff[:, :nqd], NGQ, GQ * 128)
        shared["wk"] = tile_w(w_eff[:, nqd:nqd + nkd], NGK, GK * 128)
        shared["wv"] = tile_w(w_eff[:, nqd + nkd:], NCV, VCH)
        wp = np.asarray(w_proj, np.float32)
        shared["wp"] = np.ascontiguousarray(
            wp.reshape(NQ // 2, 2, 128, NGP, gcols).transpose(0, 3, 2, 1, 4)
        ).astype(ml_dtypes.bfloat16)
        shared["bq"] = np.ascontiguousarray(
            b_eff[:nqd].reshape(NQ, 128).T.astype(np.float32))
        shared["bk"] = np.ascontiguousarray(
            b_eff[nqd:nqd + nkd].reshape(NKV, 128).T.astype(np.float32))
        shared["bv"] = b_eff[nqd + nkd:].reshape(1, nkd).astype(np.float32)
        shared["bp"] = np.asarray(b_proj, np.float32).reshape(1, H)
        shared["inv_freq"] = (
            1.0 / (BASE ** (np.arange(0, D, 2, dtype=np.float32) / D)))
        shared["ident"] = np.eye(128, dtype=ml_dtypes.bfloat16)

    s, h = c // 2, c % 2
    qpos = np.arange(h * RQ, h * RQ + RQ, dtype=np.float32)
    x_c = np.ascontiguousarray(
        np.asarray(hidden, np.float32)[s * S + h * RQ:s * S + (h + 1) * RQ]
    ).astype(ml_dtypes.bfloat16)
    ivf = shared["inv_freq"][:, None]
    scale = float(D) ** -0.5
    ang_q = ivf * qpos[None, :]
    # keys in global order; per-core causality data
    kglob = np.arange(S)
    qglob = h * RQ + np.arange(RQ)
    mask = (kglob[:, None] <= qglob[None, :])
    ebias = np.where(kglob > qglob[-1], MASK_NEG, 0.0).astype(np.float32)
    return dict(
        x=x_c,
        wq=shared["wq"], wk=shared["wk"], wv=shared["wv"], wp=shared["wp"],
        bq=shared["bq"], bk=shared["bk"], bv=shared["bv"], bp=shared["bp"],
        cq=(np.cos(ang_q) * scale).astype(np.float32),
        sq=(np.sin(ang_q) * scale).astype(np.float32),
        ck=np.cos(ang_q).astype(np.float32),
        sk=np.sin(ang_q).astype(np.float32),
        mask=mask.astype(ml_dtypes.bfloat16),
        ebias=np.ascontiguousarray(
            ebias.reshape(NT_K, 128).T.astype(np.float32)),
        ident=shared["ident"],
    )


_NC_CACHE = {}


def _get_nc(cfg_key, cfg):
    if cfg_key not in _NC_CACHE:
        _NC_CACHE[cfg_key] = build_bass(cfg)
    return _NC_CACHE[cfg_key]


def kernel(hidden_states, cu_seqlens, max_seqlen, ln_g, ln_b, w_qkv, b_qkv,
           w_proj, b_proj):
    global LAST_EXEC_NS, LAST_RESULT
    cfg = CFG_FULL
    H, S, B = cfg["H"], cfg["S"], cfg["B"]
    T = B * S
    RQ = S // 2
    assert hidden_states.shape == (T, H)
    ncores = 2 * B

    shared = {}
    in_maps = [
        prep_core_inputs(cfg, c, hidden_states, ln_g, ln_b, w_qkv, b_qkv,
                         w_proj, b_proj, shared)
        for c in range(ncores)
    ]
    nc = _get_nc("full", cfg)
    res = run_bass_kernel_spmd(
        nc, in_maps, core_ids=list(range(ncores)),
        trace=bool(os.environ.get("BASS_TRACE")),
    )
    LAST_EXEC_NS = res.exec_time_ns
    LAST_RESULT = res
    out = np.empty((T, H), np.float32)
    for c in range(ncores):
        s, h = c // 2, c % 2
        r0 = s * S + h * RQ
        out[r0:r0 + RQ] = res.results[c]["out"]
    return out


# revision 26
# speedup vs baseline: 1.0292x; 1.0292x over previous
"""Causal self-attention layer (LN + QKV + RoPE + GQA attention + proj) on 8 trn2 cores.

Sharding: sequence-parallel with pairwise K/V exchange. 8 cores = 4 packed
sequences x 2 query-halves. Core c=(s,h) owns query rows [h*512, h*512+512)
of sequence s: it normalizes and projects only its own 512 tokens, then the
pair of cores of each sequence AllGathers K/V halves (global key order) so
each core attends over the full sequence. Causality is enforced by per-core
mask data: an additive exp-bias column zeroes fully-masked key rows and
narrow leading-window mask multiplies handle the diagonal triangles, so the
single SPMD program is uniform across cores.

All matmuls run in bf16 with fp32 PSUM accumulation. Weights are pre-tiled on
the host (two k-steps per DMA) so every weight DMA is one contiguous block.
The xn transpose runs on the PE (is_transpose matmul). The softmax
denominator uses a [128,128] ones stationary so its matmul output is already
partition-broadcast.
"""

import os
import sys
import numpy as np

try:
    import concourse.bass as bass  # noqa: F401
except Exception:  # pragma: no cover
    for p in ("/opt/trn_rl_repo", "/root/.axon_site/_ro/trn_rl_repo"):
        if os.path.isdir(p) and p not in sys.path:
            sys.path.insert(0, p)

import ml_dtypes
import concourse.bass as bass
import concourse.tile as tile
from concourse import bacc, mybir
from concourse.bass_utils import run_bass_kernel_spmd

F32 = mybir.dt.float32
BF16 = mybir.dt.bfloat16

CFG_FULL = dict(H=4096, NQ=32, NKV=8, D=128, S=1024, B=4)
BASE = 10000.0
EPS = 1e-5
MASK_NEG = -30000.0

LAST_EXEC_NS = None
LAST_RESULT = None


def _ceil_div(a, b):
    return (a + b - 1) // b


def _geom(cfg):
    H, NQ, NKV, D, S = cfg["H"], cfg["NQ"], cfg["NKV"], cfg["D"], cfg["S"]
    g = {}
    g["RQ"] = S // 2
    g["RK"] = S
    g["HT"] = H // 128
    g["NT_K"] = S // 128
    g["NT_Q"] = g["RQ"] // 128
    g["NT_O"] = g["RQ"] // 128       # own token tiles
    g["VC"] = NKV * D
    g["REP"] = NQ // NKV
    g["GQ"] = 4
    g["NGQ"] = NQ // g["GQ"]
    g["GK"] = 4
    g["NGK"] = NKV // g["GK"]
    g["VCH"] = min(512, g["VC"])
    g["NCV"] = g["VC"] // g["VCH"]
    g["gcols"] = 512
    g["NGP"] = H // g["gcols"]
    return g


def build_bass(cfg):
    """Build the single-core SPMD program (identical across cores)."""
    H, NQ, NKV, D, S = cfg["H"], cfg["NQ"], cfg["NKV"], cfg["D"], cfg["S"]
    assert D == 128
    g = _geom(cfg)
    RQ, RK, HT, NT_K, NT_Q = g["RQ"], g["RK"], g["HT"], g["NT_K"], g["NT_Q"]
    NT_O = g["NT_O"]
    VC, REP = g["VC"], g["REP"]
    GQ, NGQ, GK, NGK = g["GQ"], g["NGQ"], g["GK"], g["NGK"]
    VCH, NCV, gcols, NGP = g["VCH"], g["NCV"], g["gcols"], g["NGP"]
    assert HT >= NQ and RK >= RQ
    HT2, NQ2 = HT // 2, NQ // 2

    nc = bacc.Bacc(None, target_bir_lowering=False, num_devices=8)

    x_d = nc.dram_tensor("x", [RQ, H], BF16, kind="ExternalInput")
    # weights pre-tiled on host: [k-pair, group, 128, 2, cols] so one DMA
    # loads two k-steps of one group as contiguous 2KB partition lines
    wq_d = nc.dram_tensor("wq", [HT2, NGQ, 128, 2, GQ * 128], BF16,
                          kind="ExternalInput")
    wk_d = nc.dram_tensor("wk", [HT2, NGK, 128, 2, GK * 128], BF16,
                          kind="ExternalInput")
    wv_d = nc.dram_tensor("wv", [HT2, NCV, 128, 2, VCH], BF16,
                          kind="ExternalInput")
    wp_d = nc.dram_tensor("wp", [NQ2, NGP, 128, 2, gcols], BF16,
                          kind="ExternalInput")
    bq_d = nc.dram_tensor("bq", [128, NQ], F32, kind="ExternalInput")
    bk_d = nc.dram_tensor("bk", [128, NKV], F32, kind="ExternalInput")
    bv_d = nc.dram_tensor("bv", [1, VC], F32, kind="ExternalInput")
    bp_d = nc.dram_tensor("bp", [1, H], F32, kind="ExternalInput")
    cq_d = nc.dram_tensor("cq", [64, RQ], F32, kind="ExternalInput")
    sq_d = nc.dram_tensor("sq", [64, RQ], F32, kind="ExternalInput")
    ck_d = nc.dram_tensor("ck", [64, RQ], F32, kind="ExternalInput")
    sk_d = nc.dram_tensor("sk", [64, RQ], F32, kind="ExternalInput")
    mask_d = nc.dram_tensor("mask", [RK, RQ], BF16, kind="ExternalInput")
    ident_d = nc.dram_tensor("ident", [128, 128], BF16, kind="ExternalInput")
    out_d = nc.dram_tensor("out", [RQ, H], F32, kind="ExternalOutput")

    PAIRS = [[0, 1], [2, 3], [4, 5], [6, 7]]

    def dmae(i):
        # alternate DMA issue between the two HWDGE engines
        return nc.sync if i % 2 == 0 else nc.scalar

    n_sub = _ceil_div(H, 512)
    sub = H // n_sub
    assert sub * n_sub == H and sub <= 512

    with tile.TileContext(nc) as tc:
        with (
            tc.tile_pool(name="const", bufs=1) as const,
            tc.tile_pool(name="wstream", bufs=3) as wstream,
            tc.tile_pool(name="dram", bufs=1, space="DRAM") as dram,
        ):
            # ---- constants ----
            cq_sb = const.tile([64, RQ], F32, tag="cq")
            sq_sb = const.tile([64, RQ], F32, tag="sq")
            nc.sync.dma_start(out=cq_sb[:], in_=cq_d[:])
            nc.scalar.dma_start(out=sq_sb[:], in_=sq_d[:])
            bq_sb = const.tile([128, NQ], F32, tag="bq")
            bk_sb = const.tile([128, NKV], F32, tag="bk")
            nc.sync.dma_start(out=bq_sb[:], in_=bq_d[:])
            nc.scalar.dma_start(out=bk_sb[:], in_=bk_d[:])
            ident_sb = const.tile([128, 128], BF16, tag="ident")
            nc.sync.dma_start(out=ident_sb[:], in_=ident_d[:])
            ones_blk = const.tile([128, 128], BF16, tag="ones_blk")
            nc.vector.memset(ones_blk[:], 1.0)
            eps_sb = const.tile([128, 1], F32, tag="eps")
            nc.vector.memset(eps_sb[:], EPS)

            # DRAM bounce buffers for the pairwise K/V AllGather
            kv_src = dram.tile([128, 2 * NKV, 512], BF16, tag="kv_src")
            kv_dst = dram.tile([2, 128, 2 * NKV, 512], BF16, tag="kv_dst")

            with tc.tile_pool(name="xnt_pool", bufs=1) as xnt_pool:
                xnT = xnt_pool.tile([128, HT, RQ], BF16, tag="xnT")

                with tc.tile_pool(name="qkvout", bufs=1) as qkvout:
                    QT = qkvout.tile([128, NQ, RQ], BF16, tag="QT")
                    KT = qkvout.tile([128, NKV, RK], BF16, tag="KT")
                    Vn = qkvout.tile([128, NT_K, VC], BF16, tag="Vn")

                    # ---- LN + PE-transpose of the own 512 tokens ----
                    with (
                        tc.tile_pool(name="ln", bufs=2) as ln_pool,
                        tc.tile_pool(name="stat", bufs=3) as stat,
                        tc.tile_pool(name="tps", bufs=2, space="PSUM") as tps,
                    ):
                        for tt in range(NT_O):
                            xt = ln_pool.tile([128, H], BF16, tag="xt")
                            dmae(tt).dma_start(
                                out=xt[:], in_=x_d[tt * 128:(tt + 1) * 128, :])
                            stats = stat.tile([128, n_sub, 6], F32,
                                              tag="stats")
                            xt3 = xt[:].rearrange("p (n f) -> p n f", f=sub)
                            for si in range(n_sub):
                                nc.vector.bn_stats(out=stats[:, si, :],
                                                   in_=xt3[:, si, :])
                            mv = stat.tile([128, 2], F32, tag="mv")
                            nc.vector.bn_aggr(out=mv[:], in_=stats[:])
                            rstd = stat.tile([128, 1], F32, tag="rstd")
                            nc.scalar.activation(
                                out=rstd[:], in_=mv[:, 1:2],
                                func=mybir.ActivationFunctionType.Sqrt,
                                bias=eps_sb[:], scale=1.0,
                            )
                            nc.vector.reciprocal(out=rstd[:], in_=rstd[:])
                            xnt = ln_pool.tile([128, H], BF16, tag="xnt")
                            nc.vector.tensor_scalar(
                                out=xnt[:], in0=xt[:],
                                scalar1=mv[:, 0:1], scalar2=rstd[:],
                                op0=mybir.AluOpType.subtract,
                                op1=mybir.AluOpType.mult,
                            )
                            for hg in range(HT // 4):
                                tp = tps.tile([128, 4, 128], BF16, tag="tp")
                                for i in range(4):
                                    ht = hg * 4 + i
                                    nc.tensor.transpose(
                                        tp[:, i, :],
                                        xnt[:, ht * 128:(ht + 1) * 128],
                                        ident_sb[:],
                                    )
                                nc.scalar.copy(
                                    out=xnT[:, hg * 4:(hg + 1) * 4,
                                            tt * 128:(tt + 1) * 128],
                                    in_=tp[:],
                                )

                    # One PSUM pool spans K, V and Q: all three use the same
                    # [128,512] tile shape, so the tag rotation flows across
                    # phase boundaries with no pool-transition drain.
                    kvq_ctx = tc.tile_pool(name="ps_kvq", bufs=8,
                                           space="PSUM")
                    ps_kvq = kvq_ctx.__enter__()

                    # -- K (own tokens): KT[:, :, 0:512] --
                    with (
                        tc.tile_pool(name="ropek", bufs=2) as ropek,
                        tc.tile_pool(name="ropet2", bufs=1) as ropet2,
                    ):
                        ck_sb = ropek.tile([64, RQ], F32, tag="ck", bufs=1)
                        sk_sb = ropek.tile([64, RQ], F32, tag="sk", bufs=1)
                        nc.sync.dma_start(out=ck_sb[:], in_=ck_d[:])
                        nc.scalar.dma_start(out=sk_sb[:], in_=sk_d[:])

                        def rope_apply(dst, lo, hi, cos_sb, sin_sb, tpool):
                            t1 = tpool.tile([64, 512], F32, tag="t1")
                            t2 = tpool.tile([64, 512], F32, tag="t2")
                            nc.vector.tensor_mul(t1[:], hi[:], sin_sb[:])
                            nc.vector.tensor_mul(t2[:], lo[:], cos_sb[:])
                            nc.vector.tensor_sub(dst[0:64, :], t2[:], t1[:])
                            t3 = tpool.tile([64, 512], F32, tag="t1")
                            t4 = tpool.tile([64, 512], F32, tag="t2")
                            nc.vector.tensor_mul(t3[:], lo[:], sin_sb[:])
                            nc.vector.tensor_mul(t4[:], hi[:], cos_sb[:])
                            nc.vector.tensor_add(dst[64:128, :], t4[:], t3[:])

                        def pevac(psum_ap, bias_col, lo, hi):
                            # evacuate psum halves; hi half realigned to
                            # partition 0 (two-SBUF-input ops need equal
                            # input base partitions)
                            nc.scalar.activation(
                                out=lo[:], in_=psum_ap[0:64, :],
                                func=mybir.ActivationFunctionType.Identity,
                                bias=bias_col[0:64], scale=1.0,
                            )
                            nc.scalar.activation(
                                out=hi[:], in_=psum_ap[64:128, :],
                                func=mybir.ActivationFunctionType.Identity,
                                bias=bias_col[64:128], scale=1.0,
                            )

                        for gidx in range(NGK):
                            psk = [ps_kvq.tile([128, RQ], F32, tag="ps",
                                               name=f"psk{gi}")
                                   for gi in range(GK)]
                            for k2 in range(HT2):
                                wb = wstream.tile([128, 2, GK * 128],
                                                  BF16, tag="wk")
                                dmae(k2).dma_start(
                                    out=wb[:], in_=wk_d[k2, gidx])
                                for j in range(2):
                                    for gi in range(GK):
                                        nc.tensor.matmul(
                                            psk[gi][:],
                                            wb[:, j, gi * 128:(gi + 1) * 128],
                                            xnT[:, k2 * 2 + j, :],
                                            start=(k2 == 0 and j == 0),
                                            stop=(k2 == HT2 - 1 and j == 1),
                                        )
                            for gi in range(GK):
                                h = gidx * GK + gi
                                klo = ropek.tile([64, RQ], F32, tag="klo")
                                khi = ropek.tile([64, RQ], F32, tag="khi")
                                pevac(psk[gi][:], bk_sb[:, h:h + 1], klo, khi)
                                rope_apply(KT[:, h, 0:RQ], klo, khi,
                                           ck_sb, sk_sb, ropet2)

                    # -- V (own tokens): Vn[:, 0:4, :] --
                    with tc.tile_pool(name="vb", bufs=1) as vb:
                        bv_sb = vb.tile([128, VC], F32, tag="bv")
                        nc.gpsimd.dma_start(
                            out=bv_sb[:],
                            in_=bass.AP(tensor=bv_d, offset=0,
                                        ap=[[0, 128], [1, VC]]),
                        )
                        for vch in range(NCV):
                            c0 = vch * VCH
                            psv = [ps_kvq.tile([128, VCH], F32, tag="ps",
                                               name=f"psv{ti}")
                                   for ti in range(NT_O)]
                            for k2 in range(HT2):
                                wb = wstream.tile([128, 2, VCH], BF16,
                                                  tag="wv")
                                dmae(k2).dma_start(
                                    out=wb[:], in_=wv_d[k2, vch])
                                for j in range(2):
                                    for tt in range(NT_O):
                                        nc.tensor.matmul(
                                            psv[tt][:],
                                            xnT[:, k2 * 2 + j,
                                                tt * 128:(tt + 1) * 128],
                                            wb[:, j, :],
                                            start=(k2 == 0 and j == 0),
                                            stop=(k2 == HT2 - 1 and j == 1),
                                        )
                            for tt in range(NT_O):
                                nc.vector.scalar_tensor_tensor(
                                    out=Vn[:, tt, c0:c0 + VCH],
                                    in0=psv[tt][:], scalar=1.0,
                                    in1=bv_sb[:, c0:c0 + VCH],
                                    op0=mybir.AluOpType.mult,
                                    op1=mybir.AluOpType.add,
                                )

                    # -- stage own K/V and AllGather with the pair core --
                    nc.sync.dma_start(out=kv_src[:, 0:NKV, :],
                                      in_=KT[:, :, 0:RQ])
                    nc.scalar.dma_start(
                        out=kv_src[:, NKV:2 * NKV, :],
                        in_=Vn[:, 0:NT_O, :].rearrange(
                            "p a (b c) -> p (a b) c", c=512),
                    )
                    nc.gpsimd.collective_compute(
                        "AllGather",
                        mybir.AluOpType.bypass,
                        replica_groups=PAIRS,
                        ins=[kv_src[:].opt()],
                        outs=[kv_dst[:].opt()],
                    )
                    # load back in global key order (uniform across cores);
                    # issued on gpsimd so the HWDGE queues stay free for the
                    # Q-phase weight stream below
                    nc.gpsimd.dma_start(out=KT[:, :, 0:RQ],
                                        in_=kv_dst[0, :, 0:NKV, :])
                    nc.gpsimd.dma_start(out=KT[:, :, RQ:RK],
                                        in_=kv_dst[1, :, 0:NKV, :])
                    nc.gpsimd.dma_start(
                        out=Vn[:, 0:NT_O, :].rearrange(
                            "p a (b c) -> p (a b) c", c=512),
                        in_=kv_dst[0, :, NKV:2 * NKV, :])
                    nc.gpsimd.dma_start(
                        out=Vn[:, NT_O:NT_K, :].rearrange(
                            "p a (b c) -> p (a b) c", c=512),
                        in_=kv_dst[1, :, NKV:2 * NKV, :])

                    # -- Q: QT[h] = wq[:,h].T @ xnT -- (hides the collective)
                    with (
                        tc.tile_pool(name="ropeq", bufs=2) as ropeq,
                        tc.tile_pool(name="ropet", bufs=1) as ropet,
                    ):
                        for gidx in range(NGQ):
                            psq = [ps_kvq.tile([128, RQ], F32, tag="ps",
                                               name=f"psq{gi}")
                                   for gi in range(GQ)]
                            for k2 in range(HT2):
                                wb = wstream.tile([128, 2, GQ * 128],
                                                  BF16, tag="wq")
                                dmae(k2).dma_start(
                                    out=wb[:], in_=wq_d[k2, gidx])
                                for j in range(2):
                                    for gi in range(GQ):
                                        nc.tensor.matmul(
                                            psq[gi][:],
                                            wb[:, j, gi * 128:(gi + 1) * 128],
                                            xnT[:, k2 * 2 + j, :],
                                            start=(k2 == 0 and j == 0),
                                            stop=(k2 == HT2 - 1 and j == 1),
                                        )
                            for gi in range(GQ):
                                h = gidx * GQ + gi
                                qlo = ropeq.tile([64, RQ], F32, tag="qlo")
                                qhi = ropeq.tile([64, RQ], F32, tag="qhi")
                                pevac(psq[gi][:], bq_sb[:, h:h + 1],
                                      qlo, qhi)
                                rope_apply(QT[:, h, :], qlo, qhi,
                                           cq_sb, sq_sb, ropet)
                    kvq_ctx.__exit__(None, None, None)

                    # ---- attention per q head ----
                    # attnT[h] lives in xnT's dead space: xnT[:, h, :]
                    with (
                        tc.tile_pool(name="att", bufs=3) as att,
                        tc.tile_pool(name="small", bufs=2) as small,
                        tc.tile_pool(name="msk", bufs=1) as msk,
                        tc.tile_pool(name="ps_s", bufs=2,
                                     space="PSUM") as ps_s,
                        tc.tile_pool(name="ps_o", bufs=2,
                                     space="PSUM") as ps_o,
                        tc.tile_pool(name="ps_d", bufs=2,
                                     space="PSUM") as ps_d,
                    ):
                        mask_sb = msk.tile([128, NT_K, RQ], BF16, tag="mask")
                        nc.sync.dma_start(
                            out=mask_sb[:],
                            in_=mask_d[:].rearrange("(t p) q -> p t q", p=128),
                        )
                        for h in range(NQ):
                            gkv = h // REP
                            et = att.tile([128, NT_K, RQ], BF16, tag="expT")
                            for k2 in range(NT_K // 2):
                                sps = ps_s.tile([128, 2, RQ], F32, tag="s")
                                for j in range(2):
                                    kt = k2 * 2 + j
                                    nc.tensor.matmul(
                                        sps[:, j, :],
                                        KT[:, gkv, kt * 128:(kt + 1) * 128],
                                        QT[:, h, :],
                                        start=True, stop=True,
                                    )
                                nc.scalar.activation(
                                    out=et[:, k2 * 2:k2 * 2 + 2, :],
                                    in_=sps[:],
                                    func=mybir.ActivationFunctionType.Exp,
                                )
                            for kt in range(NT_K):
                                # leading-window mask: covers this key tile's
                                # diagonal triangle and fully-masked region
                                # (mask data is all-ones where a core has no
                                # triangle here)
                                w = (kt + 1) * 128 if kt < NT_O else RQ
                                nc.vector.tensor_mul(
                                    et[:, kt, 0:w], et[:, kt, 0:w],
                                    mask_sb[:, kt, 0:w])
                            ops_ = ps_o.tile([128, RQ], F32, tag="o")
                            for kt in range(NT_K):
                                nc.tensor.matmul(
                                    ops_[:],
                                    Vn[:, kt, gkv * D:(gkv + 1) * D],
                                    et[:, kt, :],
                                    start=(kt == 0), stop=(kt == NT_K - 1),
                                )
                            # denominator via a [128,128] ones stationary:
                            # every output row is the same column sum, so the
                            # partition broadcast is built in
                            dps = ps_d.tile([128, RQ], F32, tag="d")
                            for kt in range(NT_K):
                                nc.tensor.matmul(
                                    dps[:],
                                    ones_blk[:],
                                    et[:, kt, :],
                                    start=(kt == 0), stop=(kt == NT_K - 1),
                                )
                            # the iterative-divide DVE op is the attention
                            # bottleneck in fp32; bf16 halves its cost and
                            # the denominator only needs ~3 digits
                            dsb = small.tile([128, RQ], BF16, tag="dsb")
                            rbc = small.tile([128, RQ], BF16, tag="rbc")
                            with nc.allow_low_precision(
                                    reason="softmax denom needs ~3 digits"):
                                nc.scalar.copy(out=dsb[:], in_=dps[:])
                                nc.vector.reciprocal(out=rbc[:], in_=dsb[:])
                            nc.vector.tensor_mul(
                                xnT[:, h, :], ops_[:], rbc[:])

                # ---- out = attnT.T @ wp + bp ----
                with (
                    tc.tile_pool(name="late", bufs=1) as late,
                    tc.tile_pool(name="outp", bufs=3) as outp,
                    tc.tile_pool(name="ps_c", bufs=8, space="PSUM") as ps_c,
                ):
                    bp_sb = late.tile([128, H], F32, tag="bp")
                    nc.gpsimd.dma_start(
                        out=bp_sb[:],
                        in_=bass.AP(tensor=bp_d, offset=0,
                                    ap=[[0, 128], [1, H]]),
                    )
                    for gp in range(NGP):
                        n0 = gp * gcols
                        psc = [ps_c.tile([128, gcols], F32, tag="c",
                                         name=f"psc{qt}")
                               for qt in range(NT_Q)]
                        for k2 in range(NQ2):
                            wb = wstream.tile([128, 2, gcols], BF16, tag="wp")
                            dmae(k2).dma_start(out=wb[:], in_=wp_d[k2, gp])
                            for j in range(2):
                                for qt in range(NT_Q):
                                    nc.tensor.matmul(
                                        psc[qt][:],
                                        xnT[:, k2 * 2 + j,
                                            qt * 128:(qt + 1) * 128],
                                        wb[:, j, :],
                                        start=(k2 == 0 and j == 0),
                                        stop=(k2 == NQ2 - 1 and j == 1),
                                    )
                        for qt in range(NT_Q):
                            ot = outp.tile([128, gcols], F32, tag="ot")
                            nc.vector.scalar_tensor_tensor(
                                out=ot[:], in0=psc[qt][:], scalar=1.0,
                                in1=bp_sb[:, n0:n0 + gcols],
                                op0=mybir.AluOpType.mult,
                                op1=mybir.AluOpType.add,
                            )
                            dmae(gp + qt).dma_start(
                                out=out_d[qt * 128:(qt + 1) * 128,
                                          n0:n0 + gcols],
                                in_=ot[:],
                            )

    nc.finalize()  # bacc register allocation; the pjrt path serializes as-is
    return nc


def prep_core_inputs(cfg, c, hidden, ln_g, ln_b, w_qkv, b_qkv, w_proj, b_proj,
                     shared):
    """Per-core input dict. `shared` caches the weight prep across cores."""
    H, NQ, NKV, D, S = cfg["H"], cfg["NQ"], cfg["NKV"], cfg["D"], cfg["S"]
    g = _geom(cfg)
    RQ, NT_K = g["RQ"], g["NT_K"]
    if not shared:
        ln_g = np.asarray(ln_g, np.float32)
        ln_b = np.asarray(ln_b, np.float32)
        w_qkv = np.asarray(w_qkv, np.float32)
        b_qkv = np.asarray(b_qkv, np.float32)
        w_eff = ln_g[:, None] * w_qkv
        b_eff = b_qkv + ln_b @ w_qkv
        nqd, nkd = NQ * D, NKV * D
        HT, GQ, NGQ = g["HT"], g["GQ"], g["NGQ"]
        GK, NGK, VCH, NCV = g["GK"], g["NGK"], g["VCH"], g["NCV"]
        gcols, NGP = g["gcols"], g["NGP"]

        def tile_w(w, groups, gw):
            # [H, cols] -> [HT/2, groups, 128, 2, gw] contiguous blocks
            return np.ascontiguousarray(
                w.reshape(HT // 2, 2, 128, groups, gw).transpose(0, 3, 2, 1, 4)
            ).astype(ml_dtypes.bfloat16)

        shared["wq"] = tile_w(w_eff[:, :nqd], NGQ, GQ * 128)
        shared["wk"] = tile_w(w_eff[:, nqd:nqd + nkd], NGK, GK * 128)
        shared["wv"] = tile_w(w_eff[:, nqd + nkd:], NCV, VCH)
        wp = np.asarray(w_proj, np.float32)
        shared["wp"] = np.ascontiguousarray(
            wp.reshape(NQ // 2, 2, 128, NGP, gcols).transpose(0, 3, 2, 1, 4)
        ).astype(ml_dtypes.bfloat16)
        shared["bq"] = np.ascontiguousarray(
            b_eff[:nqd].reshape(NQ, 128).T.astype(np.float32))
        shared["bk"] = np.ascontiguousarray(
            b_eff[nqd:nqd + nkd].reshape(NKV, 128).T.astype(np.float32))
        shared["bv"] = b_eff[nqd + nkd:].reshape(1, nkd).astype(np.float32)
        shared["bp"] = np.asarray(b_proj, np.float32).reshape(1, H)
        shared["inv_freq"] = (
            1.0 / (BASE ** (np.arange(0, D, 2, dtype=np.float32) / D)))
        shared["ident"] = np.eye(128, dtype=ml_dtypes.bfloat16)

    s, h = c // 2, c % 2
    qpos = np.arange(h * RQ, h * RQ + RQ, dtype=np.float32)
    x_c = np.ascontiguousarray(
        np.asarray(hidden, np.float32)[s * S + h * RQ:s * S + (h + 1) * RQ]
    ).astype(ml_dtypes.bfloat16)
    ivf = shared["inv_freq"][:, None]
    scale = float(D) ** -0.5
    ang_q = ivf * qpos[None, :]
    # keys in global order; per-core causality data
    kglob = np.arange(S)
    qglob = h * RQ + np.arange(RQ)
    mask = (kglob[:, None] <= qglob[None, :])
    return dict(
        x=x_c,
        wq=shared["wq"], wk=shared["wk"], wv=shared["wv"], wp=shared["wp"],
        bq=shared["bq"], bk=shared["bk"], bv=shared["bv"], bp=shared["bp"],
        cq=(np.cos(ang_q) * scale).astype(np.float32),
        sq=(np.sin(ang_q) * scale).astype(np.float32),
        ck=np.cos(ang_q).astype(np.float32),
        sk=np.sin(ang_q).astype(np.float32),
        mask=mask.astype(ml_dtypes.bfloat16),
        ident=shared["ident"],
    )


_NC_CACHE = {}


def _get_nc(cfg_key, cfg):
    if cfg_key not in _NC_CACHE:
        _NC_CACHE[cfg_key] = build_bass(cfg)
    return _NC_CACHE[cfg_key]


def kernel(hidden_states, cu_seqlens, max_seqlen, ln_g, ln_b, w_qkv, b_qkv,
           w_proj, b_proj):
    global LAST_EXEC_NS, LAST_RESULT
    cfg = CFG_FULL
    H, S, B = cfg["H"], cfg["S"], cfg["B"]
    T = B * S
    RQ = S // 2
    assert hidden_states.shape == (T, H)
    ncores = 2 * B

    shared = {}
    in_maps = [
        prep_core_inputs(cfg, c, hidden_states, ln_g, ln_b, w_qkv, b_qkv,
                         w_proj, b_proj, shared)
        for c in range(ncores)
    ]
    nc = _get_nc("full", cfg)
    res = run_bass_kernel_spmd(
        nc, in_maps, core_ids=list(range(ncores)),
        trace=bool(os.environ.get("BASS_TRACE")),
    )
    LAST_EXEC_NS = res.exec_time_ns
    LAST_RESULT = res
    out = np.empty((T, H), np.float32)
    for c in range(ncores):
        s, h = c // 2, c % 2
        r0 = s * S + h * RQ
        out[r0:r0 + RQ] = res.results[c]["out"]
    return out
